# revision 9
# baseline (speedup 1.0000x reference)
"""DeepseekV2 MLA prefill kernel for 8 Trainium2 NeuronCores.

Sharding strategy (from the TP hint, adapted):
  Launch A: sequence-parallel fused qkv_a projection + RMSNorms.
            Each core handles T/8 = 256 tokens, computing the transposed
            (feature-major) normed q_a / kv_a and raw k_pe for its slice.
  Host:     gather token slices -> full feature-major activations.
  Launch B: tensor-parallel over heads (2 heads/core): q_b / kv_b
            projections (ColumnParallel), rope, causal attention,
            o_proj (RowParallel) producing partial outputs.
  Host:     sum the 8 partial outputs (the RowParallel all-reduce).

Matmuls run in float32r (fp32 data on the fast single-pass PE path;
1 cycle/row when the moving free dim >= 256). Operands feeding a
float32r matmul must themselves be float32r-typed (walrus rounding
rule), so matmul-feeding tiles/weights are allocated as float32r.
"""

import numpy as np
import concourse.bacc as bacc
import concourse.tile as tile
from concourse import mybir
from concourse import bass_utils

F32 = mybir.dt.float32
F32R = mybir.dt.float32r
AF = mybir.ActivationFunctionType
AX = mybir.AxisListType

NCORES = 8
T, HID, H = 2048, 5120, 16
NOPE, ROPE, VDIM = 128, 64, 128
QLORA, KVLORA = 1536, 512
FUSED = QLORA + KVLORA + ROPE  # 2112
TS = T // NCORES               # 256 tokens/core in launch A
HPC = H // NCORES              # 2 heads/core in launch B
EPS = 1e-6
THETA = 10000.0
SCALING = float((NOPE + ROPE) ** -0.5)
NEG = -1.0e30
KT = HID // 128                # 40
MT = (FUSED + 127) // 128      # 17 (last tile 64 rows)
TT = T // 128                  # 16
NCH = T // 512                 # 4


def _build_a():
    nc = bacc.Bacc("TRN2", target_bir_lowering=False, debug=False,
                   num_devices=NCORES)
    hid_s = nc.dram_tensor("hid_s", [TS, HID], F32, kind="ExternalInput").ap()
    w_fused = nc.dram_tensor("w_fused", [HID, FUSED], F32R,
                             kind="ExternalInput").ap()
    q_ln = nc.dram_tensor("q_ln", [QLORA], F32, kind="ExternalInput").ap()
    kv_ln = nc.dram_tensor("kv_ln", [KVLORA], F32, kind="ExternalInput").ap()
    ident = nc.dram_tensor("ident", [128, 128], F32, kind="ExternalInput").ap()
    q_aT_s = nc.dram_tensor("q_aT_s", [QLORA, TS], F32,
                            kind="ExternalOutput").ap()
    kv_aT_s = nc.dram_tensor("kv_aT_s", [KVLORA, TS], F32,
                             kind="ExternalOutput").ap()
    k_peT_s = nc.dram_tensor("k_peT_s", [ROPE, TS], F32,
                             kind="ExternalOutput").ap()

    with tile.TileContext(nc) as tc:
        with tc.tile_pool(name="consts", bufs=1) as consts, \
             tc.tile_pool(name="hidT_pool", bufs=1) as hidT_pool, \
             tc.tile_pool(name="qkv_pool", bufs=1) as qkv_pool, \
             tc.tile_pool(name="small", bufs=1) as small:
            ident_sb = consts.tile([128, 128], F32)
            nc.sync.dma_start(out=ident_sb, in_=ident)
            ones_f32 = consts.tile([128, 1], F32)
            nc.vector.memset(ones_f32, 1.0)
            ones_col = consts.tile([128, 1], F32R)
            nc.vector.tensor_copy(ones_col, ones_f32)
            ones_row_f32 = consts.tile([1, 128], F32)
            nc.vector.memset(ones_row_f32, 1.0)
            ones_row = consts.tile([1, 128], F32R)
            nc.vector.tensor_copy(ones_row, ones_row_f32)
            ln_sb = consts.tile([128, 16], F32)
            nc.sync.dma_start(out=ln_sb[:, 0:12],
                              in_=q_ln.rearrange("(a p) -> p a", p=128))
            nc.sync.dma_start(out=ln_sb[:, 12:16],
                              in_=kv_ln.rearrange("(a p) -> p a", p=128))
            eps_sb = small.tile([1, 1], F32)
            nc.vector.memset(eps_sb, EPS)

            # hidT[p, k, t] = hidden_slice[t, k*128+p]
            hidT = hidT_pool.tile([128, KT, TS], F32R)
            with tc.tile_pool(name="hload", bufs=2) as hp, \
                 tc.tile_pool(name="tpsum", bufs=4, space="PSUM") as tp:
                for tt in range(TS // 128):
                    ht = hp.tile([128, HID], F32, tag="ht")
                    nc.sync.dma_start(out=ht,
                                      in_=hid_s[tt * 128:(tt + 1) * 128, :])
                    for k in range(KT):
                        ps = tp.tile([128, 128], F32, tag="tps")
                        nc.tensor.transpose(ps, ht[:, k * 128:(k + 1) * 128],
                                            ident_sb)
                        nc.vector.tensor_copy(
                            hidT[:, k, tt * 128:(tt + 1) * 128], ps)

            qkvT = qkv_pool.tile([128, MT, TS], F32)
            with tc.tile_pool(name="wpool", bufs=3) as wp, \
                 tc.tile_pool(name="mpsum", bufs=2, space="PSUM") as mp, \
                 tc.tile_pool(name="sqpool", bufs=3) as sqp, \
                 tc.tile_pool(name="sumps", bufs=1, space="PSUM") as sums_pool:
                sq_ps_q = sums_pool.tile([1, TS], F32, tag="sq_q")
                sq_ps_kv = sums_pool.tile([1, TS], F32, tag="sq_kv")
                for m in range(MT):
                    mm = min(128, FUSED - m * 128)
                    wt = wp.tile([128, KT, 128], F32R, tag="wt")
                    nc.sync.dma_start(
                        out=wt[:, :, :mm],
                        in_=w_fused[:, m * 128:m * 128 + mm].rearrange(
                            "(kt p) m -> p kt m", p=128))
                    ps = mp.tile([128, TS], F32, tag="ps")
                    for k in range(KT):
                        nc.tensor.matmul(ps[:mm], wt[:, k, :mm],
                                         hidT[:, k, :],
                                         start=(k == 0), stop=(k == KT - 1))
                    nc.vector.tensor_copy(qkvT[:mm, m, :], ps[:mm])
                    if m < 16:
                        sq = sqp.tile([128, TS], F32R, tag="sq")
                        nc.scalar.square(sq, ps)
                        tgt = sq_ps_q if m < 12 else sq_ps_kv
                        nc.tensor.matmul(tgt, ones_col, sq,
                                         start=(m in (0, 12)),
                                         stop=(m in (11, 15)),
                                         skip_group_check=True)

                # rsqrt(mean(x^2)+eps) = 1/sqrt(sumsq/D + eps)
                rq = small.tile([1, TS], F32, tag="rq")
                nc.scalar.activation(rq, sq_ps_q, func=AF.Sqrt,
                                     scale=1.0 / QLORA, bias=eps_sb)
                nc.vector.reciprocal(rq, rq)
                rq_r = small.tile([1, TS], F32R, tag="rq_r")
                nc.vector.tensor_copy(rq_r, rq)
                rkv = small.tile([1, TS], F32, tag="rkv")
                nc.scalar.activation(rkv, sq_ps_kv, func=AF.Sqrt,
                                     scale=1.0 / KVLORA, bias=eps_sb)
                nc.vector.reciprocal(rkv, rkv)
                rkv_r = small.tile([1, TS], F32R, tag="rkv_r")
                nc.vector.tensor_copy(rkv_r, rkv)
                # broadcast [1,TS] -> [128,TS] via ones-matmul (K=1)
                bq_ps = sums_pool.tile([128, TS], F32, tag="bq")
                nc.tensor.matmul(bq_ps, ones_row, rq_r, start=True, stop=True)
                bkv_ps = sums_pool.tile([128, TS], F32, tag="bkv")
                nc.tensor.matmul(bkv_ps, ones_row, rkv_r, start=True,
                                 stop=True)
                for m in range(16):
                    b = bq_ps if m < 12 else bkv_ps
                    nc.vector.tensor_mul(qkvT[:, m, :], qkvT[:, m, :], b)
                    nc.vector.tensor_scalar_mul(qkvT[:, m, :], qkvT[:, m, :],
                                                ln_sb[:, m:m + 1])

            nc.sync.dma_start(
                out=q_aT_s.rearrange("(mt p) t -> p mt t", p=128),
                in_=qkvT[:, 0:12, :])
            nc.sync.dma_start(
                out=kv_aT_s.rearrange("(mt p) t -> p mt t", p=128),
                in_=qkvT[:, 12:16, :])
            nc.sync.dma_start(out=k_peT_s, in_=qkvT[0:ROPE, 16, :])
    nc.compile()
    return nc


def _build_b():
    nc = bacc.Bacc("TRN2", target_bir_lowering=False, debug=False,
                   num_devices=NCORES)
    q_aT = nc.dram_tensor("q_aT", [QLORA, T], F32R, kind="ExternalInput").ap()
    kv_aT = nc.dram_tensor("kv_aT", [KVLORA, T], F32R,
                           kind="ExternalInput").ap()
    k_peT = nc.dram_tensor("k_peT", [ROPE, T], F32R,
                           kind="ExternalInput").ap()
    w_qb_s = nc.dram_tensor("w_qb_s", [QLORA, HPC * (NOPE + ROPE)], F32R,
                            kind="ExternalInput").ap()
    # w_kvb_s host layout: cols = [h0 nope, h1 nope, h0 v, h1 v]
    w_kvb_s = nc.dram_tensor("w_kvb_s", [KVLORA, HPC * (NOPE + VDIM)], F32R,
                             kind="ExternalInput").ap()
    w_o_s = nc.dram_tensor("w_o_s", [HPC * VDIM, HID], F32R,
                           kind="ExternalInput").ap()
    cos2 = nc.dram_tensor("cos2", [128, T], F32, kind="ExternalInput").ap()
    sin2 = nc.dram_tensor("sin2", [128, T], F32, kind="ExternalInput").ap()
    swap2t = nc.dram_tensor("swap2t", [128, 128], F32R,
                            kind="ExternalInput").ap()
    ident = nc.dram_tensor("ident", [128, 128], F32R,
                           kind="ExternalInput").ap()
    diagm = nc.dram_tensor("diagm", [128, 128], F32, kind="ExternalInput").ap()
    o_part = nc.dram_tensor("o_part", [T, HID], F32,
                            kind="ExternalOutput").ap()

    with tile.TileContext(nc) as tc:
        with tc.tile_pool(name="consts", bufs=1) as consts, \
             tc.tile_pool(name="attn_out", bufs=1) as attn_out:
            ident_sb = consts.tile([128, 128], F32R)
            nc.sync.dma_start(out=ident_sb, in_=ident)
            swap_sb = consts.tile([128, 128], F32R)
            nc.sync.dma_start(out=swap_sb, in_=swap2t)
            diag_sb = consts.tile([128, 128], F32)
            nc.sync.dma_start(out=diag_sb, in_=diagm)
            attnT = attn_out.tile([128, HPC, T], F32R)

            with tc.tile_pool(name="qk", bufs=1) as qk:
                qn_t = qk.tile([128, HPC, T], F32R)   # q nope, per head
                qpe = qk.tile([128, T], F32R)         # q pe stacked h0|h1
                qpe_ro = qk.tile([128, T], F32R)
                qpe_ro_h1 = qk.tile([64, T], F32R)    # h1 rows rebased to p0
                kn_t = qk.tile([128, HPC, T], F32R)   # k nope, per head
                kpe_raw = qk.tile([64, T], F32R)
                kpe_ro = qk.tile([64, T], F32R)
                vt = qk.tile([128, TT, HPC * VDIM], F32R)  # v token-major
                nc.sync.dma_start(out=kpe_raw, in_=k_peT)

                # ---- q_b projection -> feature-major q
                with tc.tile_pool(name="qa_p", bufs=2) as qa_p, \
                     tc.tile_pool(name="wqb_p", bufs=1) as wqb_p, \
                     tc.tile_pool(name="qpsum", bufs=3, space="PSUM") as qps:
                    wqb = wqb_p.tile([128, 12, 384], F32R)
                    nc.sync.dma_start(
                        out=wqb,
                        in_=w_qb_s.rearrange("(kt p) m -> p kt m", p=128))
                    for n in range(NCH):
                        ncol = slice(n * 512, (n + 1) * 512)
                        qa_n = qa_p.tile([128, 12, 512], F32R, tag="qa")
                        for k in range(12):
                            nc.sync.dma_start(
                                out=qa_n[:, k, :],
                                in_=q_aT[k * 128:(k + 1) * 128, ncol])
                        for m in range(3):
                            ps = qps.tile([128, 512], F32, tag="qmm")
                            for k in range(12):
                                nc.tensor.matmul(
                                    ps, wqb[:, k, m * 128:(m + 1) * 128],
                                    qa_n[:, k, :],
                                    start=(k == 0), stop=(k == 11))
                            if m == 0:
                                nc.vector.tensor_copy(qn_t[:, 0, ncol], ps)
                            elif m == 1:
                                nc.vector.tensor_copy(qpe[0:64, ncol],
                                                      ps[0:64])
                                nc.vector.tensor_copy(qn_t[0:64, 1, ncol],
                                                      ps[64:128])
                            else:
                                nc.vector.tensor_copy(qn_t[64:128, 1, ncol],
                                                      ps[0:64])
                                nc.vector.tensor_copy(qpe[64:128, ncol],
                                                      ps[64:128])

                # ---- kv_b projection -> feature-major k_nope + token-major v
                with tc.tile_pool(name="kva_p", bufs=2) as kva_p, \
                     tc.tile_pool(name="wkvb_p", bufs=1) as wkvb_p, \
                     tc.tile_pool(name="kvpsum", bufs=3, space="PSUM") as kvps:
                    wkvb = wkvb_p.tile([128, 4, 512], F32R)
                    nc.sync.dma_start(
                        out=wkvb,
                        in_=w_kvb_s.rearrange("(kt p) m -> p kt m", p=128))
                    for n in range(NCH):
                        ncol = slice(n * 512, (n + 1) * 512)
                        kva_n = kva_p.tile([128, 4, 512], F32R, tag="kva")
                        for k in range(4):
                            nc.sync.dma_start(
                                out=kva_n[:, k, :],
                                in_=kv_aT[k * 128:(k + 1) * 128, ncol])
                        for h in range(HPC):
                            ps = kvps.tile([128, 512], F32, tag="knmm")
                            for k in range(4):
                                nc.tensor.matmul(
                                    ps, wkvb[:, k, h * 128:(h + 1) * 128],
                                    kva_n[:, k, :],
                                    start=(k == 0), stop=(k == 3))
                            nc.vector.tensor_copy(kn_t[:, h, ncol], ps)
                        for stl in range(4):
                            st = n * 4 + stl
                            ps = kvps.tile([128, 256], F32, tag="vmm")
                            for k in range(4):
                                nc.tensor.matmul(
                                    ps,
                                    kva_n[:, k, stl * 128:(stl + 1) * 128],
                                    wkvb[:, k, 256:512],
                                    start=(k == 0), stop=(k == 3))
                            nc.vector.tensor_copy(vt[:, st, :], ps)

                # ---- rope (interleaved): ro = x*cos + swap(x)*sin
                with tc.tile_pool(name="ropec", bufs=1) as ropec, \
                     tc.tile_pool(name="rps", bufs=3, space="PSUM") as rps:
                    cos_sb = ropec.tile([128, T], F32)
                    nc.sync.dma_start(out=cos_sb, in_=cos2)
                    sin_sb = ropec.tile([128, T], F32)
                    nc.sync.dma_start(out=sin_sb, in_=sin2)
                    for n in range(NCH):
                        ncol = slice(n * 512, (n + 1) * 512)
                        ps = rps.tile([128, 512], F32, tag="swq")
                        nc.tensor.matmul(ps, swap_sb, qpe[:, ncol],
                                         start=True, stop=True)
                        nc.vector.tensor_mul(qpe_ro[:, ncol], qpe[:, ncol],
                                             cos_sb[:, ncol])
                        nc.vector.tensor_mul(ps, ps, sin_sb[:, ncol])
                        nc.vector.tensor_add(qpe_ro[:, ncol], qpe_ro[:, ncol],
                                             ps)
                        ps2 = rps.tile([64, 512], F32, tag="swk")
                        nc.tensor.matmul(ps2, swap_sb[0:64, 0:64],
                                         kpe_raw[:, ncol],
                                         start=True, stop=True)
                        nc.vector.tensor_mul(kpe_ro[:, ncol],
                                             kpe_raw[:, ncol],
                                             cos_sb[0:64, ncol])
                        nc.vector.tensor_mul(ps2, ps2, sin_sb[0:64, ncol])
                        nc.vector.tensor_add(kpe_ro[:, ncol], kpe_ro[:, ncol],
                                             ps2)
                        nc.vector.tensor_copy(qpe_ro_h1[:, ncol],
                                              qpe_ro[64:128, ncol])

                # ---- causal attention, 2 heads
                with tc.tile_pool(name="pT", bufs=1) as ptp, \
                     tc.tile_pool(name="prb", bufs=6) as prp, \
                     tc.tile_pool(name="smp", bufs=4) as smp, \
                     tc.tile_pool(name="scps", bufs=4, space="PSUM") as scps, \
                     tc.tile_pool(name="tps", bufs=2, space="PSUM") as tps, \
                     tc.tile_pool(name="pvps", bufs=2, space="PSUM") as pvps:
                    probst = ptp.tile([128, TT, 512], F32R)
                    for h in range(HPC):
                        for c in range(NCH):
                            for tl in range(4):
                                tt = c * 4 + tl
                                tcol = slice(tt * 128, (tt + 1) * 128)
                                nsc = tt // 4 + 1
                                sums = smp.tile([128, 4], F32, tag="sums")
                                prs = []
                                for sc in range(nsc):
                                    scol = slice(sc * 512, (sc + 1) * 512)
                                    ps = scps.tile([128, 512], F32, tag="sc")
                                    nc.tensor.matmul(ps, qn_t[:, h, tcol],
                                                     kn_t[:, h, scol],
                                                     start=True, stop=False)
                                    qpe_l = (qpe_ro[0:64, tcol] if h == 0
                                             else qpe_ro_h1[:, tcol])
                                    nc.tensor.matmul(ps, qpe_l,
                                                     kpe_ro[:, scol],
                                                     start=False, stop=True)
                                    if sc == tt // 4:
                                        d = tt * 128 - sc * 512
                                        nc.vector.tensor_add(
                                            ps[:, d:d + 128],
                                            ps[:, d:d + 128], diag_sb)
                                        if d + 128 < 512:
                                            nc.vector.memset(
                                                ps[:, d + 128:512], NEG)
                                    pr = prp.tile([128, 512], F32R, tag="pr")
                                    nc.scalar.activation(
                                        pr, ps, func=AF.Exp, scale=SCALING,
                                        accum_out=sums[:, sc:sc + 1])
                                    prs.append((sc, pr))
                                rt = smp.tile([128, 1], F32, tag="rt")
                                nc.vector.reduce_sum(rt, sums[:, 0:nsc],
                                                     axis=AX.X)
                                nc.vector.reciprocal(rt, rt)
                                for sc, pr in prs:
                                    nc.vector.tensor_scalar_mul(pr, pr, rt)
                                    for b in range(4):
                                        ps2 = tps.tile([128, 128], F32R,
                                                       tag="tr")
                                        nc.tensor.transpose(
                                            ps2,
                                            pr[:, b * 128:(b + 1) * 128],
                                            ident_sb)
                                        nc.vector.tensor_copy(
                                            probst[:, sc * 4 + b,
                                                   tl * 128:(tl + 1) * 128],
                                            ps2)
                            # PV
                            pv = pvps.tile([128, 512], F32, tag="pv")
                            ns_t = 4 * (c + 1)
                            for st in range(ns_t):
                                nc.tensor.matmul(
                                    pv, vt[:, st, h * 128:(h + 1) * 128],
                                    probst[:, st, :],
                                    start=(st == 0), stop=(st == ns_t - 1))
                            nc.vector.tensor_copy(
                                attnT[:, h, c * 512:(c + 1) * 512], pv)

            # ---- o_proj partial (RowParallel shard)
            with tc.tile_pool(name="wo_p", bufs=1) as wo_p, \
                 tc.tile_pool(name="out_p", bufs=2) as out_p, \
                 tc.tile_pool(name="ops", bufs=3, space="PSUM") as ops:
                wo = wo_p.tile([128, HPC, HID], F32R)
                nc.sync.dma_start(
                    out=wo, in_=w_o_s.rearrange("(kt p) m -> p kt m", p=128))
                for tt in range(TT):
                    tcol = slice(tt * 128, (tt + 1) * 128)
                    ob = out_p.tile([128, HID], F32, tag="ob")
                    for nch in range(HID // 512):
                        ps = ops.tile([128, 512], F32, tag="op")
                        for h in range(HPC):
                            nc.tensor.matmul(
                                ps, attnT[:, h, tcol],
                                wo[:, h, nch * 512:(nch + 1) * 512],
                                start=(h == 0), stop=(h == HPC - 1))
                        nc.vector.tensor_copy(
                            ob[:, nch * 512:(nch + 1) * 512], ps)
                    nc.sync.dma_start(out=o_part[tt * 128:(tt + 1) * 128, :],
                                      in_=ob)
    nc.compile()
    return nc


_CACHE = {}


def _get(name):
    if name not in _CACHE:
        _CACHE[name] = _build_a() if name == "a" else _build_b()
    return _CACHE[name]


def _host_consts():
    ident = np.eye(128, dtype=np.float32)
    # swap matrix S: (Sx)[2i] = -x[2i+1], (Sx)[2i+1] = x[2i]; we pass S^T,
    # block-diag over the two 64-row head slots.
    st64 = np.zeros((64, 64), dtype=np.float32)
    for i in range(32):
        st64[2 * i, 2 * i + 1] = 1.0
        st64[2 * i + 1, 2 * i] = -1.0
    swap2t = np.zeros((128, 128), dtype=np.float32)
    swap2t[0:64, 0:64] = st64
    swap2t[64:128, 64:128] = st64
    r = np.arange(128)
    diagm = np.where(r[None, :] <= r[:, None], 0.0, NEG).astype(np.float32)
    return ident, swap2t, diagm


def _rope_tables(positions):
    # duplicated-pair (interleaved) layout, rows stacked twice for 2 heads
    inv_freq = 1.0 / (THETA ** (np.arange(0, ROPE, 2, dtype=np.float32)
                                / ROPE))
    freqs = positions.astype(np.float32)[:, None] * inv_freq[None, :]  # [T,32]
    cos = np.cos(freqs).astype(np.float32)
    sin = np.sin(freqs).astype(np.float32)
    cos_dup = np.repeat(cos, 2, axis=1).T.copy()   # [64, T]
    sin_dup = np.repeat(sin, 2, axis=1).T.copy()
    cos2 = np.vstack([cos_dup, cos_dup])           # [128, T]
    sin2 = np.vstack([sin_dup, sin_dup])
    return np.ascontiguousarray(cos2), np.ascontiguousarray(sin2)


def kernel(positions, hidden_states, w_fused, q_a_ln_w, kv_a_ln_w,
           w_qb, w_kvb, w_o):
    positions = np.asarray(positions)
    hidden_states = np.ascontiguousarray(np.asarray(hidden_states,
                                                    dtype=np.float32))
    w_fused = np.ascontiguousarray(np.asarray(w_fused, dtype=np.float32))
    q_a_ln_w = np.ascontiguousarray(np.asarray(q_a_ln_w, dtype=np.float32))
    kv_a_ln_w = np.ascontiguousarray(np.asarray(kv_a_ln_w, dtype=np.float32))
    w_qb = np.asarray(w_qb, dtype=np.float32)
    w_kvb = np.asarray(w_kvb, dtype=np.float32)
    w_o = np.asarray(w_o, dtype=np.float32)

    ident, swap2t, diagm = _host_consts()
    cos2, sin2 = _rope_tables(positions)

    # ---- launch A: sequence-parallel fused projection + norms
    nca = _get("a")
    in_a = []
    for c in range(NCORES):
        in_a.append({
            "hid_s": np.ascontiguousarray(
                hidden_states[c * TS:(c + 1) * TS, :]),
            "w_fused": w_fused,
            "q_ln": q_a_ln_w,
            "kv_ln": kv_a_ln_w,
            "ident": ident,
        })
    res_a = bass_utils.run_bass_kernel_spmd(nca, in_a,
                                            core_ids=list(range(NCORES)))
    q_aT = np.concatenate([res_a.results[c]["q_aT_s"]
                           for c in range(NCORES)], axis=1)
    kv_aT = np.concatenate([res_a.results[c]["kv_aT_s"]
                            for c in range(NCORES)], axis=1)
    k_peT = np.concatenate([res_a.results[c]["k_peT_s"]
                            for c in range(NCORES)], axis=1)

    # ---- launch B: head-parallel attention
    ncb = _get("b")
    in_b = []
    for c in range(NCORES):
        g0, g1 = 2 * c, 2 * c + 1
        wq_s = np.ascontiguousarray(
            w_qb[:, g0 * (NOPE + ROPE):(g1 + 1) * (NOPE + ROPE)])
        wk = w_kvb
        wkv_s = np.ascontiguousarray(np.concatenate([
            wk[:, g0 * 256:g0 * 256 + 128],        # h0 nope
            wk[:, g1 * 256:g1 * 256 + 128],        # h1 nope
            wk[:, g0 * 256 + 128:(g0 + 1) * 256],  # h0 v
            wk[:, g1 * 256 + 128:(g1 + 1) * 256],  # h1 v
        ], axis=1))
        wo_s = np.ascontiguousarray(w_o[g0 * VDIM:(g1 + 1) * VDIM, :])
        in_b.append({
            "q_aT": q_aT, "kv_aT": kv_aT, "k_peT": k_peT,
            "w_qb_s": wq_s, "w_kvb_s": wkv_s, "w_o_s": wo_s,
            "cos2": cos2, "sin2": sin2,
            "swap2t": swap2t, "ident": ident, "diagm": diagm,
        })
    res_b = bass_utils.run_bass_kernel_spmd(ncb, in_b,
                                            core_ids=list(range(NCORES)))
    out = res_b.results[0]["o_part"].astype(np.float64)
    for c in range(1, NCORES):
        out += res_b.results[c]["o_part"]
    return out.astype(np.float32)


# revision 10
# speedup vs baseline: 1.0070x; 1.0070x over previous
"""DeepseekV2 MLA prefill kernel for 8 Trainium2 NeuronCores.

Sharding strategy (from the TP hint, adapted):
  Launch A: sequence-parallel fused qkv_a projection + RMSNorms.
            Each core handles T/8 = 256 tokens, computing the transposed
            (feature-major) normed q_a / kv_a and raw k_pe for its slice.
  Host:     gather token slices -> full feature-major activations.
  Launch B: tensor-parallel over heads (2 heads/core): q_b / kv_b
            projections (ColumnParallel), rope, causal attention,
            o_proj (RowParallel) producing partial outputs.
  Host:     sum the 8 partial outputs (the RowParallel all-reduce).

Matmuls run in float32r (fp32 data on the fast single-pass PE path;
1 cycle/row when the moving free dim >= 256). Operands feeding a
float32r matmul must themselves be float32r-typed (walrus rounding
rule), so matmul-feeding tiles/weights are allocated as float32r.
"""

import numpy as np
import concourse.bacc as bacc
import concourse.tile as tile
from concourse import mybir
from concourse import bass_utils

F32 = mybir.dt.float32
F32R = mybir.dt.float32r
AF = mybir.ActivationFunctionType
AX = mybir.AxisListType

NCORES = 8
T, HID, H = 2048, 5120, 16
NOPE, ROPE, VDIM = 128, 64, 128
QLORA, KVLORA = 1536, 512
FUSED = QLORA + KVLORA + ROPE  # 2112
TS = T // NCORES               # 256 tokens/core in launch A
HPC = H // NCORES              # 2 heads/core in launch B
EPS = 1e-6
THETA = 10000.0
SCALING = float((NOPE + ROPE) ** -0.5)
NEG = -1.0e30
KT = HID // 128                # 40
MT = (FUSED + 127) // 128      # 17 (last tile 64 rows)
TT = T // 128                  # 16
NCH = T // 512                 # 4


def _build_a():
    nc = bacc.Bacc("TRN2", target_bir_lowering=False, debug=False,
                   num_devices=NCORES)
    hid_s = nc.dram_tensor("hid_s", [TS, HID], F32, kind="ExternalInput").ap()
    w_fused = nc.dram_tensor("w_fused", [HID, FUSED], F32R,
                             kind="ExternalInput").ap()
    q_ln = nc.dram_tensor("q_ln", [QLORA], F32, kind="ExternalInput").ap()
    kv_ln = nc.dram_tensor("kv_ln", [KVLORA], F32, kind="ExternalInput").ap()
    ident = nc.dram_tensor("ident", [128, 128], F32, kind="ExternalInput").ap()
    q_aT_s = nc.dram_tensor("q_aT_s", [QLORA, TS], F32,
                            kind="ExternalOutput").ap()
    kv_aT_s = nc.dram_tensor("kv_aT_s", [KVLORA, TS], F32,
                             kind="ExternalOutput").ap()
    k_peT_s = nc.dram_tensor("k_peT_s", [ROPE, TS], F32,
                             kind="ExternalOutput").ap()

    with tile.TileContext(nc) as tc:
        with tc.tile_pool(name="consts", bufs=1) as consts, \
             tc.tile_pool(name="hidT_pool", bufs=1) as hidT_pool, \
             tc.tile_pool(name="qkv_pool", bufs=1) as qkv_pool, \
             tc.tile_pool(name="small", bufs=1) as small:
            ident_sb = consts.tile([128, 128], F32)
            nc.sync.dma_start(out=ident_sb, in_=ident)
            ones_f32 = consts.tile([128, 1], F32)
            nc.vector.memset(ones_f32, 1.0)
            ones_col = consts.tile([128, 1], F32R)
            nc.vector.tensor_copy(ones_col, ones_f32)
            ones_row_f32 = consts.tile([1, 128], F32)
            nc.vector.memset(ones_row_f32, 1.0)
            ones_row = consts.tile([1, 128], F32R)
            nc.vector.tensor_copy(ones_row, ones_row_f32)
            ln_sb = consts.tile([128, 16], F32)
            nc.sync.dma_start(out=ln_sb[:, 0:12],
                              in_=q_ln.rearrange("(a p) -> p a", p=128))
            nc.sync.dma_start(out=ln_sb[:, 12:16],
                              in_=kv_ln.rearrange("(a p) -> p a", p=128))
            eps_sb = small.tile([1, 1], F32)
            nc.vector.memset(eps_sb, EPS)

            # hidT[p, k, t] = hidden_slice[t, k*128+p]
            hidT = hidT_pool.tile([128, KT, TS], F32R)
            with tc.tile_pool(name="hload", bufs=2) as hp, \
                 tc.tile_pool(name="tpsum", bufs=4, space="PSUM") as tp:
                for tt in range(TS // 128):
                    ht = hp.tile([128, HID], F32, tag="ht")
                    nc.sync.dma_start(out=ht,
                                      in_=hid_s[tt * 128:(tt + 1) * 128, :])
                    for k in range(KT):
                        ps = tp.tile([128, 128], F32, tag="tps")
                        nc.tensor.transpose(ps, ht[:, k * 128:(k + 1) * 128],
                                            ident_sb)
                        nc.vector.tensor_copy(
                            hidT[:, k, tt * 128:(tt + 1) * 128], ps)

            qkvT = qkv_pool.tile([128, MT, TS], F32)
            with tc.tile_pool(name="wpool", bufs=3) as wp, \
                 tc.tile_pool(name="mpsum", bufs=2, space="PSUM") as mp, \
                 tc.tile_pool(name="sqpool", bufs=3) as sqp, \
                 tc.tile_pool(name="sumps", bufs=1, space="PSUM") as sums_pool:
                sq_ps_q = sums_pool.tile([1, TS], F32, tag="sq_q")
                sq_ps_kv = sums_pool.tile([1, TS], F32, tag="sq_kv")
                for m in range(MT):
                    mm = min(128, FUSED - m * 128)
                    wt = wp.tile([128, KT, 128], F32R, tag="wt")
                    nc.sync.dma_start(
                        out=wt[:, :, :mm],
                        in_=w_fused[:, m * 128:m * 128 + mm].rearrange(
                            "(kt p) m -> p kt m", p=128))
                    ps = mp.tile([128, TS], F32, tag="ps")
                    for k in range(KT):
                        nc.tensor.matmul(ps[:mm], wt[:, k, :mm],
                                         hidT[:, k, :],
                                         start=(k == 0), stop=(k == KT - 1))
                    nc.vector.tensor_copy(qkvT[:mm, m, :], ps[:mm])
                    if m < 16:
                        sq = sqp.tile([128, TS], F32R, tag="sq")
                        nc.scalar.square(sq, ps)
                        tgt = sq_ps_q if m < 12 else sq_ps_kv
                        nc.tensor.matmul(tgt, ones_col, sq,
                                         start=(m in (0, 12)),
                                         stop=(m in (11, 15)),
                                         skip_group_check=True)

                # rsqrt(mean(x^2)+eps) = 1/sqrt(sumsq/D + eps)
                rq = small.tile([1, TS], F32, tag="rq")
                nc.scalar.activation(rq, sq_ps_q, func=AF.Sqrt,
                                     scale=1.0 / QLORA, bias=eps_sb)
                nc.vector.reciprocal(rq, rq)
                rq_r = small.tile([1, TS], F32R, tag="rq_r")
                nc.vector.tensor_copy(rq_r, rq)
                rkv = small.tile([1, TS], F32, tag="rkv")
                nc.scalar.activation(rkv, sq_ps_kv, func=AF.Sqrt,
                                     scale=1.0 / KVLORA, bias=eps_sb)
                nc.vector.reciprocal(rkv, rkv)
                rkv_r = small.tile([1, TS], F32R, tag="rkv_r")
                nc.vector.tensor_copy(rkv_r, rkv)
                # broadcast [1,TS] -> [128,TS] via ones-matmul (K=1)
                bq_ps = sums_pool.tile([128, TS], F32, tag="bq")
                nc.tensor.matmul(bq_ps, ones_row, rq_r, start=True, stop=True)
                bkv_ps = sums_pool.tile([128, TS], F32, tag="bkv")
                nc.tensor.matmul(bkv_ps, ones_row, rkv_r, start=True,
                                 stop=True)
                for m in range(16):
                    b = bq_ps if m < 12 else bkv_ps
                    nc.vector.tensor_mul(qkvT[:, m, :], qkvT[:, m, :], b)
                    nc.vector.tensor_scalar_mul(qkvT[:, m, :], qkvT[:, m, :],
                                                ln_sb[:, m:m + 1])

            nc.sync.dma_start(
                out=q_aT_s.rearrange("(mt p) t -> p mt t", p=128),
                in_=qkvT[:, 0:12, :])
            nc.sync.dma_start(
                out=kv_aT_s.rearrange("(mt p) t -> p mt t", p=128),
                in_=qkvT[:, 12:16, :])
            nc.sync.dma_start(out=k_peT_s, in_=qkvT[0:ROPE, 16, :])
    nc.compile()
    return nc


def _build_b():
    nc = bacc.Bacc("TRN2", target_bir_lowering=False, debug=False,
                   num_devices=NCORES)
    q_aT = nc.dram_tensor("q_aT", [QLORA, T], F32R, kind="ExternalInput").ap()
    kv_aT = nc.dram_tensor("kv_aT", [KVLORA, T], F32R,
                           kind="ExternalInput").ap()
    k_peT = nc.dram_tensor("k_peT", [ROPE, T], F32R,
                           kind="ExternalInput").ap()
    w_qb_s = nc.dram_tensor("w_qb_s", [QLORA, HPC * (NOPE + ROPE)], F32R,
                            kind="ExternalInput").ap()
    # w_kvb_s host layout: cols = [h0 nope, h1 nope, h0 v, h1 v]
    w_kvb_s = nc.dram_tensor("w_kvb_s", [KVLORA, HPC * (NOPE + VDIM)], F32R,
                             kind="ExternalInput").ap()
    w_o_s = nc.dram_tensor("w_o_s", [HPC * VDIM, HID], F32R,
                           kind="ExternalInput").ap()
    cos2 = nc.dram_tensor("cos2", [128, T], F32, kind="ExternalInput").ap()
    sin2 = nc.dram_tensor("sin2", [128, T], F32, kind="ExternalInput").ap()
    swap2t = nc.dram_tensor("swap2t", [128, 128], F32R,
                            kind="ExternalInput").ap()
    ident = nc.dram_tensor("ident", [128, 128], F32R,
                           kind="ExternalInput").ap()
    diagm = nc.dram_tensor("diagm", [128, 128], F32, kind="ExternalInput").ap()
    o_part = nc.dram_tensor("o_part", [T, HID], F32,
                            kind="ExternalOutput").ap()

    with tile.TileContext(nc) as tc:
        with tc.tile_pool(name="consts", bufs=1) as consts, \
             tc.tile_pool(name="attn_out", bufs=1) as attn_out:
            ident_sb = consts.tile([128, 128], F32R)
            nc.sync.dma_start(out=ident_sb, in_=ident)
            swap_sb = consts.tile([128, 128], F32R)
            nc.sync.dma_start(out=swap_sb, in_=swap2t)
            diag_sb = consts.tile([128, 128], F32)
            nc.sync.dma_start(out=diag_sb, in_=diagm)
            ones_row_f32 = consts.tile([1, 128], F32)
            nc.vector.memset(ones_row_f32, 1.0)
            ones_row = consts.tile([1, 128], F32R)
            nc.vector.tensor_copy(ones_row, ones_row_f32)
            attnT = attn_out.tile([128, HPC, T], F32R)

            with tc.tile_pool(name="qk", bufs=1) as qk:
                qn_t = qk.tile([128, HPC, T], F32R)   # q nope, per head
                qpe = qk.tile([128, T], F32R)         # q pe stacked h0|h1
                qpe_ro = qk.tile([128, T], F32R)
                qpe_ro_h1 = qk.tile([64, T], F32R)    # h1 rows rebased to p0
                kn_t = qk.tile([128, HPC, T], F32R)   # k nope, per head
                kpe_raw = qk.tile([64, T], F32R)
                kpe_ro = qk.tile([64, T], F32R)
                vt = qk.tile([128, TT, HPC * VDIM], F32R)  # v token-major
                nc.sync.dma_start(out=kpe_raw, in_=k_peT)

                # ---- q_b projection -> feature-major q
                with tc.tile_pool(name="qa_p", bufs=2) as qa_p, \
                     tc.tile_pool(name="wqb_p", bufs=1) as wqb_p, \
                     tc.tile_pool(name="qpsum", bufs=3, space="PSUM") as qps:
                    wqb = wqb_p.tile([128, 12, 384], F32R)
                    nc.sync.dma_start(
                        out=wqb,
                        in_=w_qb_s.rearrange("(kt p) m -> p kt m", p=128))
                    for n in range(NCH):
                        ncol = slice(n * 512, (n + 1) * 512)
                        qa_n = qa_p.tile([128, 12, 512], F32R, tag="qa")
                        for k in range(12):
                            nc.sync.dma_start(
                                out=qa_n[:, k, :],
                                in_=q_aT[k * 128:(k + 1) * 128, ncol])
                        for m in range(3):
                            ps = qps.tile([128, 512], F32, tag="qmm")
                            for k in range(12):
                                nc.tensor.matmul(
                                    ps, wqb[:, k, m * 128:(m + 1) * 128],
                                    qa_n[:, k, :],
                                    start=(k == 0), stop=(k == 11))
                            if m == 0:
                                nc.vector.tensor_copy(qn_t[:, 0, ncol], ps)
                            elif m == 1:
                                nc.vector.tensor_copy(qpe[0:64, ncol],
                                                      ps[0:64])
                                nc.vector.tensor_copy(qn_t[0:64, 1, ncol],
                                                      ps[64:128])
                            else:
                                nc.vector.tensor_copy(qn_t[64:128, 1, ncol],
                                                      ps[0:64])
                                nc.vector.tensor_copy(qpe[64:128, ncol],
                                                      ps[64:128])

                # ---- kv_b projection -> feature-major k_nope + token-major v
                with tc.tile_pool(name="kva_p", bufs=2) as kva_p, \
                     tc.tile_pool(name="wkvb_p", bufs=1) as wkvb_p, \
                     tc.tile_pool(name="kvpsum", bufs=3, space="PSUM") as kvps:
                    wkvb = wkvb_p.tile([128, 4, 512], F32R)
                    nc.sync.dma_start(
                        out=wkvb,
                        in_=w_kvb_s.rearrange("(kt p) m -> p kt m", p=128))
                    for n in range(NCH):
                        ncol = slice(n * 512, (n + 1) * 512)
                        kva_n = kva_p.tile([128, 4, 512], F32R, tag="kva")
                        for k in range(4):
                            nc.sync.dma_start(
                                out=kva_n[:, k, :],
                                in_=kv_aT[k * 128:(k + 1) * 128, ncol])
                        for h in range(HPC):
                            ps = kvps.tile([128, 512], F32, tag="knmm")
                            for k in range(4):
                                nc.tensor.matmul(
                                    ps, wkvb[:, k, h * 128:(h + 1) * 128],
                                    kva_n[:, k, :],
                                    start=(k == 0), stop=(k == 3))
                            nc.vector.tensor_copy(kn_t[:, h, ncol], ps)
                        for stl in range(4):
                            st = n * 4 + stl
                            ps = kvps.tile([128, 256], F32, tag="vmm")
                            for k in range(4):
                                nc.tensor.matmul(
                                    ps,
                                    kva_n[:, k, stl * 128:(stl + 1) * 128],
                                    wkvb[:, k, 256:512],
                                    start=(k == 0), stop=(k == 3))
                            nc.vector.tensor_copy(vt[:, st, :], ps)

                # ---- rope (interleaved): ro = x*cos + swap(x)*sin
                with tc.tile_pool(name="ropec", bufs=1) as ropec, \
                     tc.tile_pool(name="rps", bufs=3, space="PSUM") as rps:
                    cos_sb = ropec.tile([128, T], F32)
                    nc.sync.dma_start(out=cos_sb, in_=cos2)
                    sin_sb = ropec.tile([128, T], F32)
                    nc.sync.dma_start(out=sin_sb, in_=sin2)
                    for n in range(NCH):
                        ncol = slice(n * 512, (n + 1) * 512)
                        ps = rps.tile([128, 512], F32, tag="swq")
                        nc.tensor.matmul(ps, swap_sb, qpe[:, ncol],
                                         start=True, stop=True)
                        nc.vector.tensor_mul(qpe_ro[:, ncol], qpe[:, ncol],
                                             cos_sb[:, ncol])
                        nc.vector.tensor_mul(ps, ps, sin_sb[:, ncol])
                        nc.vector.tensor_add(qpe_ro[:, ncol], qpe_ro[:, ncol],
                                             ps)
                        ps2 = rps.tile([64, 512], F32, tag="swk")
                        nc.tensor.matmul(ps2, swap_sb[0:64, 0:64],
                                         kpe_raw[:, ncol],
                                         start=True, stop=True)
                        nc.vector.tensor_mul(kpe_ro[:, ncol],
                                             kpe_raw[:, ncol],
                                             cos_sb[0:64, ncol])
                        nc.vector.tensor_mul(ps2, ps2, sin_sb[0:64, ncol])
                        nc.vector.tensor_add(kpe_ro[:, ncol], kpe_ro[:, ncol],
                                             ps2)
                        nc.vector.tensor_copy(qpe_ro_h1[:, ncol],
                                              qpe_ro[64:128, ncol])

                # ---- causal attention, 2 heads
                with tc.tile_pool(name="pT", bufs=1) as ptp, \
                     tc.tile_pool(name="prb", bufs=6) as prp, \
                     tc.tile_pool(name="smp", bufs=4) as smp, \
                     tc.tile_pool(name="scps", bufs=3, space="PSUM") as scps, \
                     tc.tile_pool(name="tps", bufs=2, space="PSUM") as tps, \
                     tc.tile_pool(name="pvps", bufs=1, space="PSUM") as pvps, \
                     tc.tile_pool(name="rtsps", bufs=1, space="PSUM") as rtsps, \
                     tc.tile_pool(name="bcps", bufs=1, space="PSUM") as bcps:
                    probst = ptp.tile([128, TT, 512], F32R)
                    for h in range(HPC):
                        for c in range(NCH):
                            rts_ps = rtsps.tile([1, 512], F32R, tag="rts")
                            for tl in range(4):
                                tt = c * 4 + tl
                                tcol = slice(tt * 128, (tt + 1) * 128)
                                nsc = tt // 4 + 1
                                sums = smp.tile([128, 4], F32, tag="sums")
                                prs = []
                                for sc in range(nsc):
                                    scol = slice(sc * 512, (sc + 1) * 512)
                                    ps = scps.tile([128, 512], F32, tag="sc")
                                    nc.tensor.matmul(ps, qn_t[:, h, tcol],
                                                     kn_t[:, h, scol],
                                                     start=True, stop=False)
                                    qpe_l = (qpe_ro[0:64, tcol] if h == 0
                                             else qpe_ro_h1[:, tcol])
                                    nc.tensor.matmul(ps, qpe_l,
                                                     kpe_ro[:, scol],
                                                     start=False, stop=True)
                                    if sc == tt // 4:
                                        d = tt * 128 - sc * 512
                                        nc.vector.tensor_add(
                                            ps[:, d:d + 128],
                                            ps[:, d:d + 128], diag_sb)
                                        if d + 128 < 512:
                                            nc.vector.memset(
                                                ps[:, d + 128:512], NEG)
                                    pr = prp.tile([128, 512], F32R, tag="pr")
                                    nc.scalar.activation(
                                        pr, ps, func=AF.Exp, scale=SCALING,
                                        accum_out=sums[:, sc:sc + 1])
                                    prs.append((sc, pr))
                                rt = smp.tile([128, 1], F32, tag="rt")
                                nc.vector.reduce_sum(rt, sums[:, 0:nsc],
                                                     axis=AX.X)
                                nc.vector.reciprocal(rt, rt)
                                rt_r = smp.tile([128, 1], F32R, tag="rt_r")
                                nc.vector.tensor_copy(rt_r, rt)
                                nc.tensor.matmul(
                                    rts_ps[:, tl * 128:(tl + 1) * 128],
                                    rt_r, ident_sb, is_transpose=True,
                                    start=True, stop=True,
                                    skip_group_check=True)
                                for sc, pr in prs:
                                    for b in range(4):
                                        ps2 = tps.tile([128, 128], F32R,
                                                       tag="tr")
                                        nc.tensor.transpose(
                                            ps2,
                                            pr[:, b * 128:(b + 1) * 128],
                                            ident_sb)
                                        eng = (nc.vector if b % 2 == 0
                                               else nc.scalar)
                                        dst = probst[:, sc * 4 + b,
                                                     tl * 128:(tl + 1) * 128]
                                        if b % 2 == 0:
                                            nc.vector.tensor_copy(dst, ps2)
                                        else:
                                            nc.scalar.copy(dst, ps2)
                            # normalizer row: bps[p, j] = 1/rowsum(t=c*512+j)
                            rts_sb = smp.tile([1, 512], F32R, tag="rts_sb")
                            nc.vector.tensor_copy(rts_sb, rts_ps)
                            bps = bcps.tile([128, 512], F32, tag="bps")
                            nc.tensor.matmul(bps, ones_row, rts_sb,
                                             start=True, stop=True)
                            # PV
                            pv = pvps.tile([128, 512], F32, tag="pv")
                            ns_t = 4 * (c + 1)
                            for st in range(ns_t):
                                nc.tensor.matmul(
                                    pv, vt[:, st, h * 128:(h + 1) * 128],
                                    probst[:, st, :],
                                    start=(st == 0), stop=(st == ns_t - 1))
                            acol = slice(c * 512, (c + 1) * 512)
                            nc.vector.tensor_copy(attnT[:, h, acol], pv)
                            nc.vector.tensor_mul(attnT[:, h, acol],
                                                 attnT[:, h, acol], bps)

            # ---- o_proj partial (RowParallel shard)
            with tc.tile_pool(name="wo_p", bufs=1) as wo_p, \
                 tc.tile_pool(name="out_p", bufs=2) as out_p, \
                 tc.tile_pool(name="ops", bufs=3, space="PSUM") as ops:
                wo = wo_p.tile([128, HPC, HID], F32R)
                nc.sync.dma_start(
                    out=wo, in_=w_o_s.rearrange("(kt p) m -> p kt m", p=128))
                for tt in range(TT):
                    tcol = slice(tt * 128, (tt + 1) * 128)
                    ob = out_p.tile([128, HID], F32, tag="ob")
                    for nch in range(HID // 512):
                        ps = ops.tile([128, 512], F32, tag="op")
                        for h in range(HPC):
                            nc.tensor.matmul(
                                ps, attnT[:, h, tcol],
                                wo[:, h, nch * 512:(nch + 1) * 512],
                                start=(h == 0), stop=(h == HPC - 1))
                        if nch % 2 == 0:
                            nc.vector.tensor_copy(
                                ob[:, nch * 512:(nch + 1) * 512], ps)
                        else:
                            nc.scalar.copy(
                                ob[:, nch * 512:(nch + 1) * 512], ps)
                    nc.sync.dma_start(out=o_part[tt * 128:(tt + 1) * 128, :],
                                      in_=ob)
    nc.compile()
    return nc


_CACHE = {}


def _get(name):
    if name not in _CACHE:
        _CACHE[name] = _build_a() if name == "a" else _build_b()
    return _CACHE[name]


def _host_consts():
    ident = np.eye(128, dtype=np.float32)
    # swap matrix S: (Sx)[2i] = -x[2i+1], (Sx)[2i+1] = x[2i]; we pass S^T,
    # block-diag over the two 64-row head slots.
    st64 = np.zeros((64, 64), dtype=np.float32)
    for i in range(32):
        st64[2 * i, 2 * i + 1] = 1.0
        st64[2 * i + 1, 2 * i] = -1.0
    swap2t = np.zeros((128, 128), dtype=np.float32)
    swap2t[0:64, 0:64] = st64
    swap2t[64:128, 64:128] = st64
    r = np.arange(128)
    diagm = np.where(r[None, :] <= r[:, None], 0.0, NEG).astype(np.float32)
    return ident, swap2t, diagm


def _rope_tables(positions):
    # duplicated-pair (interleaved) layout, rows stacked twice for 2 heads
    inv_freq = 1.0 / (THETA ** (np.arange(0, ROPE, 2, dtype=np.float32)
                                / ROPE))
    freqs = positions.astype(np.float32)[:, None] * inv_freq[None, :]  # [T,32]
    cos = np.cos(freqs).astype(np.float32)
    sin = np.sin(freqs).astype(np.float32)
    cos_dup = np.repeat(cos, 2, axis=1).T.copy()   # [64, T]
    sin_dup = np.repeat(sin, 2, axis=1).T.copy()
    cos2 = np.vstack([cos_dup, cos_dup])           # [128, T]
    sin2 = np.vstack([sin_dup, sin_dup])
    return np.ascontiguousarray(cos2), np.ascontiguousarray(sin2)


def kernel(positions, hidden_states, w_fused, q_a_ln_w, kv_a_ln_w,
           w_qb, w_kvb, w_o):
    positions = np.asarray(positions)
    hidden_states = np.ascontiguousarray(np.asarray(hidden_states,
                                                    dtype=np.float32))
    w_fused = np.ascontiguousarray(np.asarray(w_fused, dtype=np.float32))
    q_a_ln_w = np.ascontiguousarray(np.asarray(q_a_ln_w, dtype=np.float32))
    kv_a_ln_w = np.ascontiguousarray(np.asarray(kv_a_ln_w, dtype=np.float32))
    w_qb = np.asarray(w_qb, dtype=np.float32)
    w_kvb = np.asarray(w_kvb, dtype=np.float32)
    w_o = np.asarray(w_o, dtype=np.float32)

    ident, swap2t, diagm = _host_consts()
    cos2, sin2 = _rope_tables(positions)

    # ---- launch A: sequence-parallel fused projection + norms
    nca = _get("a")
    in_a = []
    for c in range(NCORES):
        in_a.append({
            "hid_s": np.ascontiguousarray(
                hidden_states[c * TS:(c + 1) * TS, :]),
            "w_fused": w_fused,
            "q_ln": q_a_ln_w,
            "kv_ln": kv_a_ln_w,
            "ident": ident,
        })
    res_a = bass_utils.run_bass_kernel_spmd(nca, in_a,
                                            core_ids=list(range(NCORES)))
    q_aT = np.concatenate([res_a.results[c]["q_aT_s"]
                           for c in range(NCORES)], axis=1)
    kv_aT = np.concatenate([res_a.results[c]["kv_aT_s"]
                            for c in range(NCORES)], axis=1)
    k_peT = np.concatenate([res_a.results[c]["k_peT_s"]
                            for c in range(NCORES)], axis=1)

    # ---- launch B: head-parallel attention
    ncb = _get("b")
    in_b = []
    for c in range(NCORES):
        g0, g1 = 2 * c, 2 * c + 1
        wq_s = np.ascontiguousarray(
            w_qb[:, g0 * (NOPE + ROPE):(g1 + 1) * (NOPE + ROPE)])
        wk = w_kvb
        wkv_s = np.ascontiguousarray(np.concatenate([
            wk[:, g0 * 256:g0 * 256 + 128],        # h0 nope
            wk[:, g1 * 256:g1 * 256 + 128],        # h1 nope
            wk[:, g0 * 256 + 128:(g0 + 1) * 256],  # h0 v
            wk[:, g1 * 256 + 128:(g1 + 1) * 256],  # h1 v
        ], axis=1))
        wo_s = np.ascontiguousarray(w_o[g0 * VDIM:(g1 + 1) * VDIM, :])
        in_b.append({
            "q_aT": q_aT, "kv_aT": kv_aT, "k_peT": k_peT,
            "w_qb_s": wq_s, "w_kvb_s": wkv_s, "w_o_s": wo_s,
            "cos2": cos2, "sin2": sin2,
            "swap2t": swap2t, "ident": ident, "diagm": diagm,
        })
    res_b = bass_utils.run_bass_kernel_spmd(ncb, in_b,
                                            core_ids=list(range(NCORES)))
    out = res_b.results[0]["o_part"].astype(np.float64)
    for c in range(1, NCORES):
        out += res_b.results[c]["o_part"]
    return out.astype(np.float32)
